# revision 1
# baseline (speedup 1.0000x reference)
"""Trainium2 Bass kernel for nn_DES_PSP_Model (LSTM encoder + CNN + AR decoder).

Sharding: pure data parallel, batch 128 -> 8 cores x 16.

Encoder: 5-layer LSTM over T=256 run as a time wavefront (tick s computes
cell (l, s-l) for all valid l) with cross-layer batched vector ops in
[4H -> partitions, 5 layers x 16 batch -> free] layout.

Cell math (all-tanh trick): store H=2h, C=2c. Host pre-scales weights:
g-gate rows x2, h-input columns x0.5, gate chunks permuted to
chunkA=[f;i], chunkB=[o;g]. One ACT tanh(0.5*psum) gives s=tanh of all
gates; sigma(x) = 0.5(s+1). Then
  m1 = (sf+1)*C ; m2 = (si+1)*sg ; C' = 0.5*m1 + m2
  tc = tanh(0.5*C') ; H' = (so+1)*tc
Biases enter the psum via a K=6 matmul: stationary [x-row; 5 bias rows],
rhs = [x_t broadcast-slot; one-hot layer indicators].

CNN: conv0+avgpool folded (host im2col of the 1-channel input, W0/4),
conv1-7 as 9 shifted-AP matmuls (fp32r) with 2-way PE row tiling over a
partition-duplicated activation tile; ReLU+bias on ACT; GAP on DVE.

Decoder: 14 sequential steps x 5 layers, same cell. Gate bias enters the
psum via a K=2 matmul (const ones rhs) so ONE tanh ACT covers both chunks;
each cell writes H' twice (next-layer input slot + next-step state slot),
and the fc ACT writes the y feedback directly into the layer-0 rhs row.

v2 changes vs v1: per-tick x/indicator copies replaced by a precomputed
[6, 260*80+32] rhs tile; gate ACT split per chunk (m1 hides under ACT-B);
bf16 gate/elementwise tiles; uint32-bitcast shift copy.
"""
import os
import sys
import numpy as np
from contextlib import ExitStack

sys.path.insert(0, "/opt/trn_rl_repo")
os.environ.setdefault("JAX_PLATFORMS", "axon")

import ml_dtypes  # noqa: E402

BF = ml_dtypes.bfloat16

B, T, HID, L, PS = 128, 256, 64, 5, 14
ALPHA = 0.2
CNN_LAYERS = 8
NCORES = 8
BP = B // NCORES          # 16 batch per core
G4 = 4 * HID              # 256
W5 = L * BP               # 80  (5 layer slots x 16 batch)
IMG = 32                  # input image side
PM = 16                   # pooled side
PPAD = PM + 2             # 18 padded side
PIMG = PPAD * PPAD        # 324 per padded image

# pytorch gate rows: i[0:64] f[64:128] g[128:192] o[192:256]
# chunkA rows = [f; i], chunkB rows = [o; g]
_PERM_A = np.r_[64:128, 0:64]
_PERM_B = np.r_[192:256, 128:192]


# ----------------------------------------------------------------------------
# host-side weight preparation (pure layout/scale transforms)
# ----------------------------------------------------------------------------

def _gate_row_scale():
    """Row scale in chunk-permuted order: g rows x2 (chunkB bottom half)."""
    sA = np.ones(128, np.float32)
    sB = np.ones(128, np.float32)
    sB[64:128] = 2.0
    return sA, sB


def _chunk(W, perm, rowscale):
    # W: [4H, K] -> permuted+scaled chunk [128, K]
    return W[perm] * rowscale[:, None]


def _stat_kstack(Wih, Whh, perm, rowscale):
    """lhsT [128,128] for layers>=1: rows 0:64 Wih-part (h-in, x0.5),
    rows 64:128 Whh-part (x0.5)."""
    ci = _chunk(Wih, perm, rowscale) * 0.5   # [128, 64]
    ch = _chunk(Whh, perm, rowscale) * 0.5   # [128, 64]
    return np.concatenate([ci.T, ch.T], axis=0)  # [128, 128]


def prep_host(inputs):
    """Build per-core input maps (list of dicts of np arrays)."""
    x = np.asarray(inputs["x"], np.float32)
    y = np.asarray(inputs["y"], np.float32)
    f32 = lambda a: np.asarray(a, np.float32)
    enc_Wih0, enc_Wih = f32(inputs["enc_Wih0"]), f32(inputs["enc_Wih"])
    enc_Whh, enc_b = f32(inputs["enc_Whh"]), f32(inputs["enc_b"])
    dec_Wih0, dec_Wih = f32(inputs["dec_Wih0"]), f32(inputs["dec_Wih"])
    dec_Whh, dec_b = f32(inputs["dec_Whh"]), f32(inputs["dec_b"])
    fc_W, fc_b = f32(inputs["fc_W"]), f32(inputs["fc_b"])
    conv0_W, conv0_b = f32(inputs["conv0_W"]), f32(inputs["conv0_b"])
    convs_W, convs_b = f32(inputs["convs_W"]), f32(inputs["convs_b"])

    sA, sB = _gate_row_scale()

    # ---- lstmw: bf16 [128, nblocks*128 + 128] ----
    # All recurrent stationaries are K=64 [64,128] blocks in rows 0:64 so
    # matmuls read H (a [64, *] tile) directly -- no K-stack, no shift copy.
    blocks = []  # list of [128, 128] blocks (f32)

    def bias_block(Wih0, b_l):  # rows 0:6: x row + 5 per-layer bias rows
        blkA = np.zeros((128, 128), np.float32)
        blkB = np.zeros((128, 128), np.float32)
        blkA[0] = _chunk(Wih0, _PERM_A, sA)[:, 0]
        blkB[0] = _chunk(Wih0, _PERM_B, sB)[:, 0]
        for j in range(L):
            blkA[1 + j] = _chunk(b_l[j][:, None], _PERM_A, sA)[:, 0]
            blkB[1 + j] = _chunk(b_l[j][:, None], _PERM_B, sB)[:, 0]
        return blkA, blkB

    def k64(W, perm, rowscale):  # [64, 128] lhsT in rows 0:64 (H is 2h)
        blk_ = np.zeros((128, 128), np.float32)
        blk_[0:64] = (_chunk(W, perm, rowscale) * 0.5).T
        return blk_

    # encoder: 0/1 bias, 2/3 l0 Whh, then per layer 1..4:
    # WihA@(4+4(l-1)) WhhA@(5+..) WihB@(6+..) WhhB@(7+..)   -> 4..19
    eA, eB = bias_block(enc_Wih0, enc_b)
    blocks += [eA, eB]
    blocks += [k64(enc_Whh[0], _PERM_A, sA), k64(enc_Whh[0], _PERM_B, sB)]
    for l in range(1, L):
        blocks += [k64(enc_Wih[l - 1], _PERM_A, sA),
                   k64(enc_Whh[l], _PERM_A, sA),
                   k64(enc_Wih[l - 1], _PERM_B, sB),
                   k64(enc_Whh[l], _PERM_B, sB)]
    # decoder: 20/21 Wy (row 0, unscaled: y is not doubled), 22/23 l0 Whh,
    # per layer 1..4: WihA@(24+4(l-1)) WhhA WihB WhhB -> 24..39
    wyA = np.zeros((128, 128), np.float32)
    wyB = np.zeros((128, 128), np.float32)
    wyA[0] = _chunk(dec_Wih0, _PERM_A, sA)[:, 0]
    wyB[0] = _chunk(dec_Wih0, _PERM_B, sB)[:, 0]
    blocks += [wyA, wyB]
    blocks += [k64(dec_Whh[0], _PERM_A, sA), k64(dec_Whh[0], _PERM_B, sB)]
    for l in range(1, L):
        blocks += [k64(dec_Wih[l - 1], _PERM_A, sA),
                   k64(dec_Whh[l], _PERM_A, sA),
                   k64(dec_Wih[l - 1], _PERM_B, sB),
                   k64(dec_Whh[l], _PERM_B, sB)]
    # decoder bias blocks 40..44: rows 0:2 = [bA; bB] (in-psum scale, g x2)
    for l in range(L):
        bb = np.zeros((128, 128), np.float32)
        bb[0] = _chunk(dec_b[l][:, None], _PERM_A, sA)[:, 0]
        bb[1] = _chunk(dec_b[l][:, None], _PERM_B, sB)[:, 0]
        blocks.append(bb)
    lstmw = np.concatenate(blocks, axis=1)  # [128, 45*128]
    # fc block: col 45*128 holds lhsT [64,1] = (0.5*fc_W).T
    fccol = np.zeros((128, 64), np.float32)
    fccol[0:64, 0] = 0.5 * fc_W[0]
    # conv0 stationary [9, 64] at cols 24*128+1 .. +64? pack separately:
    c0 = np.zeros((128, 64), np.float32)
    for k in range(9):
        dy, dx = k // 3 - 1, k % 3 - 1
        c0[k] = conv0_W[:, 0, dy + 1, dx + 1] / 4.0
    lstmw = np.concatenate([lstmw, fccol, c0], axis=1).astype(BF)  # [128, 5888]

    # ---- cnnw: bf16 [128, 7*6*64]: uniform K=128 tap-pair stationaries ----
    # block p 0-2: rows 0:64 = tap (dy=p-1, dx=-1), rows 64:128 = tap (dy, 0)
    # block p 3-5: rows 0:64 = tap (dy=p-4, dx=+1), rows 64:128 = 0
    # (rhs bottom half is z pre-shifted by +1 column)
    cb = []
    for i in range(CNN_LAYERS - 1):
        for p in range(6):
            blk = np.zeros((128, 64), np.float32)
            if p < 3:
                dy = p - 1
                blk[0:64] = convs_W[i, :, :, dy + 1, 0].T
                blk[64:128] = convs_W[i, :, :, dy + 1, 1].T
            else:
                dy = p - 4
                blk[0:64] = convs_W[i, :, :, dy + 1, 2].T
            cb.append(blk)
    cnnw = np.concatenate(cb, axis=1).astype(BF)  # [128, 2688]

    # ---- misc: f32 [128, 32] ----
    misc = np.zeros((128, 32), np.float32)
    # decoder ACT bias (post-scale): i,f,o: 0.5*b ; g: b   (chunk-permuted)
    half = np.ones(256, np.float32) * 0.5
    half[128:192] = 1.0  # g rows (pytorch order) get 1.0
    for l in range(L):
        bb = dec_b[l] * half
        misc[:, 2 * l] = bb[_PERM_A]
        misc[:, 2 * l + 1] = bb[_PERM_B]
    misc[0, 10] = fc_b[0]
    misc[0:64, 11] = conv0_b
    for i in range(CNN_LAYERS - 1):
        misc[0:64, 12 + i] = convs_b[i]

    # ---- per-core tensors ----
    TICKS = T + L - 1
    ypad = np.pad(y[:, 0], ((0, 0), (1, 1), (1, 1)))  # [B, 34, 34]
    in_maps = []
    for c in range(NCORES):
        sl = slice(c * BP, (c + 1) * BP)
        xs = x[sl, :, 0]  # [BP, T]
        xtm = np.ascontiguousarray(xs.T).reshape(1, T * BP).astype(BF)
        # rhsx [6, TICKS*80 + 32]: per-tick bias/x rhs + decoder ones cols
        rhsx = np.zeros((6, TICKS * W5 + 2 * BP), np.float32)
        for s in range(TICKS):
            if s < T:
                rhsx[0, s * W5:s * W5 + BP] = xs[:, s]
            for j in range(L):
                rhsx[1 + j, s * W5 + j * BP:s * W5 + (j + 1) * BP] = 1.0
        # decoder bias-MM ones patterns (8-wide per half-batch pipe)
        oc = TICKS * W5
        rhsx[0, oc:oc + 8] = 1.0
        rhsx[1, oc + 8:oc + 16] = 1.0
        rhsx = rhsx.astype(BF)
        # yim2col [9, BP*1024]
        yp = ypad[sl]  # [BP, 34, 34]
        yim = np.zeros((9, BP, IMG, IMG), np.float32)
        for k in range(9):
            dy, dx = k // 3 - 1, k % 3 - 1
            yim[k] = yp[:, 1 + dy:1 + dy + IMG, 1 + dx:1 + dx + IMG]
        yim = yim.reshape(9, BP * IMG * IMG).astype(BF)
        in_maps.append(dict(
            lstmw=lstmw, cnnw=cnnw, rhsx=rhsx, misc=misc,
            x=xtm, yim=yim,
        ))
    return in_maps


# ----------------------------------------------------------------------------
# device program
# ----------------------------------------------------------------------------

_CACHE = {}


def build_program():
    import concourse.bass as bass  # noqa: F401
    import concourse.tile as tile
    from concourse import bacc, mybir

    F32 = mybir.dt.float32
    F32R = mybir.dt.float32r
    BF16 = mybir.dt.bfloat16
    AF = mybir.ActivationFunctionType
    OP = mybir.AluOpType

    TICKS = int(os.environ.get("BASSK_TICKS", T + L - 1))  # 260
    DSTEPS = int(os.environ.get("BASSK_DSTEPS", PS))
    DO_CNN = int(os.environ.get("BASSK_CNN", 1))
    NCONV = int(os.environ.get("BASSK_NCONV", CNN_LAYERS))
    DO_GAP = int(os.environ.get("BASSK_GAP", 1))

    nc = bacc.Bacc("TRN2", target_bir_lowering=False, debug=False,
                   num_devices=NCORES)
    RXW = (T + L - 1) * W5 + 2 * BP
    d_lstmw = nc.dram_tensor("lstmw", [128, 5888], BF16, kind="ExternalInput").ap()
    d_cnnw = nc.dram_tensor("cnnw", [128, 2688], BF16, kind="ExternalInput").ap()
    d_rhsx = nc.dram_tensor("rhsx", [6, RXW], BF16, kind="ExternalInput").ap()
    d_misc = nc.dram_tensor("misc", [128, 32], F32, kind="ExternalInput").ap()
    d_x = nc.dram_tensor("x", [1, T * BP], BF16, kind="ExternalInput").ap()
    d_yim = nc.dram_tensor("yim", [9, BP * IMG * IMG], BF16,
                           kind="ExternalInput").ap()
    d_out = nc.dram_tensor("out", [1, PS * BP], F32, kind="ExternalOutput").ap()

    # stationary block column offsets in lstmw
    def blk(i):
        return slice(i * 128, (i + 1) * 128)
    FC_COL = 45 * 128
    C0_COL = 45 * 128 + 64
    ONES_COL = (T + L - 1) * W5

    with tile.TileContext(nc) as tc:
        with ExitStack() as ctx:
            const = ctx.enter_context(tc.tile_pool(name="const", bufs=1))
            state = ctx.enter_context(tc.tile_pool(name="state", bufs=1))
            spool = ctx.enter_context(tc.tile_pool(name="spool", bufs=2))
            mpool = ctx.enter_context(tc.tile_pool(name="mpool", bufs=2))
            apool = ctx.enter_context(tc.tile_pool(name="apool", bufs=2))
            dpool = ctx.enter_context(tc.tile_pool(name="dpool", bufs=2))
            eps = ctx.enter_context(tc.tile_pool(name="eps", bufs=2, space="PSUM"))
            cps = ctx.enter_context(tc.tile_pool(name="cps", bufs=2, space="PSUM"))
            dps = ctx.enter_context(tc.tile_pool(name="dps", bufs=1, space="PSUM"))

            # ---- constants ----
            lw = const.tile([128, 5888], BF16, tag="lw", name="lw")
            nc.sync.dma_start(lw[:], d_lstmw)
            cw = const.tile([128, 2688], BF16, tag="cw", name="cw") if DO_CNN else None
            if DO_CNN:
                nc.sync.dma_start(cw[:], d_cnnw)
            xw = const.tile([1, T * BP], BF16, tag="xw", name="xw")
            nc.sync.dma_start(xw[:], d_x)
            yimt = const.tile([9, BP * IMG * IMG], BF16, tag="yimt", name="yimt") if DO_CNN else None
            if DO_CNN:
                nc.sync.dma_start(yimt[:], d_yim)
            misct = const.tile([128, 32], F32, tag="misct", name="misct")
            nc.sync.dma_start(misct[:], d_misc)
            rxt = const.tile([6, RXW], BF16, tag="rxt", name="rxt")
            nc.sync.dma_start(rxt[:], d_rhsx)

            # ---- persistent state ----
            # Htb[:, l*BP+b] = 2*h^l[b] (latest value for layer l)
            Htb = state.tile([64, W5], BF16, tag="H", name="H")
            Ct = state.tile([64, W5], F32, tag="C", name="C")
            ydata = state.tile([1, BP], BF16, tag="ydata", name="ydata")
            nc.gpsimd.memset(Htb[:], 0.0)
            nc.gpsimd.memset(Ct[:], 0.0)
            z2a = state.tile([128, BP * PIMG], BF16, tag="z2a", name="z2a") if DO_CNN else None
            z2b = state.tile([128, BP * PIMG], BF16, tag="z2b", name="z2b") if DO_CNN else None
            if DO_CNN:
                nc.gpsimd.memset(z2a[:], 0.0)
                nc.gpsimd.memset(z2b[:], 0.0)
            feat = state.tile([64, BP], F32, tag="feat", name="feat")
            feat2 = state.tile([64, BP], BF16, tag="feat2", name="feat2")
            outt = state.tile([1, PS * BP], F32, tag="outt", name="outt")
            if DSTEPS == 0:
                nc.gpsimd.memset(outt[:], 0.0)

            # =============== CNN emission slices ===============
            # CNN ops are emitted interleaved into the encoder tick loop in
            # small slices so PE/Scalar/Vector FIFO insertions never stall
            # the encoder's serial chain by more than ~1 op.
            cnn_slices = []
            if DO_CNN:
                c0st = lw[:, C0_COL:C0_COL + 64]  # [9 rows used, 64]
                z1v = z2a[:].rearrange("p (i r c) -> p i r c", i=BP, r=PPAD)

                def conv0_chunk(n):
                    def emit():
                        img, hh = n // 2, n % 2
                        pc = cps.tile([64, 512], F32, tag="cpg", name="cpg")
                        nc.tensor.matmul(
                            pc[:], c0st[0:9, :],
                            yimt[0:9, n * 512:(n + 1) * 512],
                            start=True, stop=True)
                        # PSUM -> SBUF copy on Scalar, pool pairs on DVE
                        pp = apool.tile([64, 512], F32, tag="poolP",
                                        name="poolP")
                        nc.scalar.activation(pp[:], pc[:], AF.Copy)
                        at = apool.tile([64, 256], F32, tag="poolA",
                                        name="poolA")
                        p4 = pp[:].rearrange("p (r c two) -> p r c two",
                                             r=16, two=2)
                        nc.vector.tensor_tensor(
                            at[:].rearrange("p (r c) -> p r c", r=16),
                            p4[:, :, :, 0], p4[:, :, :, 1], op=OP.add)
                        a4 = at[:].rearrange("p (r two c) -> p r two c",
                                             two=2, c=16)
                        nc.vector.scalar_tensor_tensor(
                            z1v[0:64, img, 1 + 8 * hh:9 + 8 * hh, 1:17],
                            a4[:, :, 0, :], misct[0:64, 11:12],
                            a4[:, :, 1, :], op0=OP.add, op1=OP.add)
                        nc.gpsimd.tensor_copy(
                            z1v[64:128, img, 1 + 8 * hh:9 + 8 * hh, 0:16],
                            z1v[0:64, img, 1 + 8 * hh:9 + 8 * hh, 1:17])
                    return emit

                for n in range(2 * BP):
                    cnn_slices.append(conv0_chunk(n))

                ccell = {}

                def conv_mms(i, n, prange, zin):
                    def emit():
                        ziv = zin[:].rearrange("p (i r c) -> p i r c",
                                               i=BP, r=PPAD)
                        if prange[0] == 0:
                            ccell['pc'] = cps.tile([64, 512], F32, tag="cpg",
                                                   name="cpg")
                        pc = ccell['pc']
                        i0 = 2 * n
                        for p in prange:
                            dy = (p - 1) if p < 3 else (p - 4)
                            c0_ = 0 if p < 3 else 2
                            st_ = cw[:, (i - 1) * 384 + p * 64:
                                     (i - 1) * 384 + p * 64 + 64]
                            rhs = ziv[:, i0:i0 + 2, 1 + dy:17 + dy,
                                      c0_:c0_ + 16]
                            nc.tensor.matmul(pc[:], st_, rhs,
                                             start=(p == 0), stop=(p == 5))
                    return emit

                def conv_relu(i, n, zout):
                    def emit():
                        pc = ccell['pc']
                        i0 = 2 * n
                        zov = zout[:].rearrange("p (i r c) -> p i r c",
                                                i=BP, r=PPAD)
                        # relu+bias on DVE (gpsimd has no PSUM port)
                        nc.vector.tensor_scalar(
                            zov[0:64, i0:i0 + 2, 1:17, 1:17],
                            pc[:].rearrange("p (i r c) -> p i r c",
                                            i=2, r=16),
                            misct[0:64, 11 + i:12 + i], 0.0,
                            op0=OP.add, op1=OP.max)
                        if i < CNN_LAYERS - 1:
                            nc.gpsimd.tensor_copy(
                                zov[64:128, i0:i0 + 2, 1:17, 0:16],
                                zov[0:64, i0:i0 + 2, 1:17, 1:17])
                    return emit

                zin, zout = z2a, z2b
                for i in range(1, NCONV):
                    for n in range(BP // 2):
                        cnn_slices.append(conv_mms(i, n, (0, 1, 2), zin))
                        cnn_slices.append(conv_mms(i, n, (3, 4, 5), zin))
                        cnn_slices.append(conv_relu(i, n, zout))
                    zin, zout = zout, zin

                if DO_GAP:
                    zfin = zin

                    def gap_j(j):
                        def emit():
                            zfv = zfin[:].rearrange("p (i r c) -> p i r c",
                                                    i=BP, r=PPAD)
                            nc.vector.tensor_reduce(
                                feat[:, j:j + 1], zfv[0:64, j, 1:17, 1:17],
                                axis=mybir.AxisListType.XY, op=OP.add)
                        return emit

                    for j in range(BP):
                        cnn_slices.append(gap_j(j))
                    cnn_slices.append(
                        lambda: nc.vector.tensor_copy(feat2[:], feat[:]))
                else:
                    cnn_slices.append(
                        lambda: nc.gpsimd.memset(feat2[:], 0.0))
            else:
                cnn_slices.append(lambda: nc.gpsimd.memset(feat2[:], 0.0))

            # =============== encoder wavefront ===============
            for s in range(TICKS):
                lmin = max(0, s - (T - 1))
                lmax = min(L - 1, s)
                lo, w = lmin * BP, (lmax - lmin + 1) * BP
                rx = rxt[0:6, s * W5 + lo:s * W5 + lo + w]

                pgA = eps.tile([128, W5], F32, tag="pgA", name="pgA")
                pgB = eps.tile([128, W5], F32, tag="pgB", name="pgB")
                for (pgc, bias_b, l0_b, lay_b) in ((pgA, 0, 2, 4),
                                                   (pgB, 1, 3, 6)):
                    nc.tensor.matmul(pgc[:, lo:lo + w], lw[0:6, blk(bias_b)],
                                     rx, start=True, stop=False)
                    if lmin == 0:
                        nc.tensor.matmul(pgc[:, 0:BP], lw[0:64, blk(l0_b)],
                                         Htb[:, 0:BP], start=False,
                                         stop=(lmax == 0))
                    for l in range(max(1, lmin), lmax + 1):
                        sl_ = slice(l * BP, (l + 1) * BP)
                        bW = blk(lay_b + 4 * (l - 1))       # Wih: reads col l-1
                        bH = blk(lay_b + 1 + 4 * (l - 1))   # Whh: reads col l
                        nc.tensor.matmul(pgc[:, sl_], lw[0:64, bW],
                                         Htb[:, (l - 1) * BP:l * BP],
                                         start=False, stop=False)
                        nc.tensor.matmul(pgc[:, sl_], lw[0:64, bH],
                                         Htb[:, l * BP:(l + 1) * BP],
                                         start=False, stop=(l == lmax))

                # gates: per-chunk tanh so chunk-A consumers start earlier
                stA = spool.tile([128, W5], BF16, tag="stA", name="stA")
                stB = spool.tile([128, W5], BF16, tag="stB", name="stB")
                nc.scalar.activation(stA[:, lo:lo + w], pgA[:, lo:lo + w],
                                     AF.Tanh, scale=0.5)
                nc.scalar.activation(stB[:, lo:lo + w], pgB[:, lo:lo + w],
                                     AF.Tanh, scale=0.5)

                m1 = mpool.tile([64, W5], F32, tag="m1", name="m1")
                m2 = mpool.tile([64, W5], BF16, tag="m2", name="m2")
                tcn = mpool.tile([64, W5], BF16, tag="tc", name="tc")
                # m1 = (sf+1)*C  (needs only chunk A; runs under ACT-B)
                nc.vector.scalar_tensor_tensor(
                    m1[:, lo:lo + w], stA[0:64, lo:lo + w], 1.0,
                    Ct[:, lo:lo + w], op0=OP.add, op1=OP.mult)
                # m2 = (si+1)*sg
                nc.vector.scalar_tensor_tensor(
                    m2[:, lo:lo + w], stA[64:128, lo:lo + w], 1.0,
                    stB[64:128, lo:lo + w], op0=OP.add, op1=OP.mult)
                # C = 0.5*m1 + m2
                nc.vector.scalar_tensor_tensor(
                    Ct[:, lo:lo + w], m1[:, lo:lo + w], 0.5,
                    m2[:, lo:lo + w], op0=OP.mult, op1=OP.add)
                # tc = tanh(0.5*C)
                nc.scalar.activation(tcn[:, lo:lo + w], Ct[:, lo:lo + w],
                                     AF.Tanh, scale=0.5)
                # H = (so+1)*tc  (single write; no shift copy needed)
                nc.vector.scalar_tensor_tensor(
                    Htb[:, lo:lo + w], stB[0:64, lo:lo + w], 1.0,
                    tcn[:, lo:lo + w], op0=OP.add, op1=OP.mult)

                # one CNN slice per tick, queued behind this tick's ops so
                # its PE/ACT/DVE work lands in the chain's idle windows
                if s < len(cnn_slices):
                    cnn_slices[s]()

            # leftover CNN slices (if any) after the encoder
            for k in range(TICKS, len(cnn_slices)):
                cnn_slices[k]()

            # =============== fuse -> decoder init ===============
            # Decoder reuses Htb[:, l*BP] = latest 2*h^l. Within a step,
            # layer l's Wih-matmul reads col l-1 (fresh h^{l-1}_t) and its
            # Whh-matmul reads col l (h^l_{t-1}), then H' overwrites col l.
            kf = 2.0 * ALPHA / 256.0
            for j in range(L):
                nc.vector.scalar_tensor_tensor(
                    Htb[:, j * BP:(j + 1) * BP], feat2[:], kf,
                    Htb[:, j * BP:(j + 1) * BP],
                    op0=OP.mult, op1=OP.add)
            nc.vector.tensor_copy(ydata[0:1, :],
                                  xw[0:1, (T - 1) * BP:T * BP])

            # =============== decoder (two half-batch pipes) ===============
            # Two independent 8-batch chains interleave on the engines so
            # each pipe's serial latency hides under the other's work.
            HB = BP // 2
            ones8 = rxt[0:2, ONES_COL:ONES_COL + 2 * HB]
            for step in range(DSTEPS):
                for l in range(L):
                    pds, sds = [], []
                    for pipe in range(2):
                        po = pipe * HB
                        c0 = l * BP + po
                        pd = dps.tile([128, 2 * HB], F32, tag=f"dpg{pipe}",
                                      name=f"dpg{pipe}")
                        nc.tensor.matmul(pd[:, 0:2 * HB],
                                         lw[0:2, blk(40 + l)],
                                         ones8, start=True, stop=False)
                        if l == 0:
                            ysl = ydata[0:1, po:po + HB]
                            nc.tensor.matmul(pd[:, 0:HB], lw[0:1, blk(20)],
                                             ysl, start=False, stop=False)
                            nc.tensor.matmul(pd[:, HB:2 * HB],
                                             lw[0:1, blk(21)],
                                             ysl, start=False, stop=False)
                            nc.tensor.matmul(pd[:, 0:HB], lw[0:64, blk(22)],
                                             Htb[:, c0:c0 + HB],
                                             start=False, stop=False)
                            nc.tensor.matmul(pd[:, HB:2 * HB],
                                             lw[0:64, blk(23)],
                                             Htb[:, c0:c0 + HB],
                                             start=False, stop=True)
                        else:
                            b0 = 24 + 4 * (l - 1)
                            p0 = (l - 1) * BP + po
                            nc.tensor.matmul(pd[:, 0:HB], lw[0:64, blk(b0)],
                                             Htb[:, p0:p0 + HB],
                                             start=False, stop=False)
                            nc.tensor.matmul(pd[:, 0:HB],
                                             lw[0:64, blk(b0 + 1)],
                                             Htb[:, c0:c0 + HB],
                                             start=False, stop=False)
                            nc.tensor.matmul(pd[:, HB:2 * HB],
                                             lw[0:64, blk(b0 + 2)],
                                             Htb[:, p0:p0 + HB],
                                             start=False, stop=False)
                            nc.tensor.matmul(pd[:, HB:2 * HB],
                                             lw[0:64, blk(b0 + 3)],
                                             Htb[:, c0:c0 + HB],
                                             start=False, stop=True)
                        pds.append(pd)
                    for pipe in range(2):
                        sd = dpool.tile([128, 2 * HB], BF16,
                                        tag=f"sdec{pipe}", name=f"sdec{pipe}")
                        nc.scalar.activation(sd[:], pds[pipe][:], AF.Tanh,
                                             scale=0.5)
                        sds.append(sd)
                    dtcs = []
                    for pipe in range(2):
                        po = pipe * HB
                        c0 = l * BP + po
                        sd = sds[pipe]
                        dm1 = mpool.tile([64, HB], F32, tag=f"dm1{pipe}",
                                         name=f"dm1{pipe}")
                        dm2 = mpool.tile([64, HB], BF16, tag=f"dm2{pipe}",
                                         name=f"dm2{pipe}")
                        nc.vector.scalar_tensor_tensor(
                            dm1[:], sd[0:64, 0:HB], 1.0, Ct[:, c0:c0 + HB],
                            op0=OP.add, op1=OP.mult)
                        nc.vector.scalar_tensor_tensor(
                            dm2[:], sd[64:128, 0:HB], 1.0,
                            sd[64:128, HB:2 * HB], op0=OP.add, op1=OP.mult)
                        nc.vector.scalar_tensor_tensor(
                            Ct[:, c0:c0 + HB], dm1[:], 0.5, dm2[:],
                            op0=OP.mult, op1=OP.add)
                    for pipe in range(2):
                        po = pipe * HB
                        c0 = l * BP + po
                        dtc = mpool.tile([64, HB], BF16, tag=f"dtc{pipe}",
                                         name=f"dtc{pipe}")
                        nc.scalar.activation(dtc[:], Ct[:, c0:c0 + HB],
                                             AF.Tanh, scale=0.5)
                        dtcs.append(dtc)
                        nc.vector.scalar_tensor_tensor(
                            Htb[:, c0:c0 + HB], sds[pipe][0:64, HB:2 * HB],
                            1.0, dtc[:], op0=OP.add, op1=OP.mult)
                # fc + output; y feedback written straight into ydata
                for pipe in range(2):
                    po = pipe * HB
                    pf = dps.tile([128, 2 * HB], F32, tag=f"dpg{pipe}",
                                  name=f"dpg{pipe}")
                    nc.tensor.matmul(pf[0:1, 0:HB],
                                     lw[0:64, FC_COL:FC_COL + 1],
                                     Htb[:, (L - 1) * BP + po:
                                         (L - 1) * BP + po + HB],
                                     start=True, stop=True)
                    if step + 1 < DSTEPS:
                        nc.scalar.activation(ydata[0:1, po:po + HB],
                                             pf[0:1, 0:HB], AF.Identity,
                                             bias=misct[0:1, 10:11])
                    nc.scalar.activation(
                        outt[0:1, step * BP + po:step * BP + po + HB],
                        pf[0:1, 0:HB], AF.Identity, bias=misct[0:1, 10:11])

            nc.sync.dma_start(d_out, outt[:])

    nc.compile()
    return nc


def kernel(**inputs) -> np.ndarray:
    from concourse.bass_utils import run_bass_kernel_spmd
    if "nc" not in _CACHE:
        _CACHE["nc"] = build_program()
    nc = _CACHE["nc"]
    in_maps = prep_host(inputs)
    res = run_bass_kernel_spmd(nc, in_maps, list(range(NCORES)))
    outs = []
    for c in range(NCORES):
        o = np.asarray(res.results[c]["out"], np.float32).reshape(PS, BP)
        outs.append(o.T[:, :, None])  # [BP, PS, 1]
    return np.concatenate(outs, axis=0)

